# revision 3
# baseline (speedup 1.0000x reference)
"""CRF negative-log-likelihood kernel for Trainium2 (8 NeuronCores, Bass/Tile).

Strategy
--------
Data-parallel over batch: each of the 8 cores gets 32 of the 256 sequences
plus a replicated copy of the tiny (50,50) transition matrix.

The forward algorithm runs in *linear* space: with
    E   = exp(trans[:48,:48])            (fixed 48x48 matrix)
    f_t = exp(feat_t - c)                (c = constant drift-centering bias)
the log-space recurrence becomes  alpha_t = (alpha_{t-1} @ E) * f_t,
i.e. per step one tiny PE matmul (stationary E bf16, moving [48,32] bf16)
plus one DVE tensor_tensor multiply (PSUM x SBUF -> SBUF bf16).  The
constant bias c keeps the whole trajectory inside fp32 range; the dropped
scale is restored on the host as T*c per sequence.

Two INDEPENDENT chains run interleaved and meet in the middle: the forward
chain from t=0 (stationary E) and the backward chain from t=T-1
(stationary E^T), pipelined against each other so each chain's
PE<->DVE round-trip latency is hidden behind the other's engine work.
total_score = alpha_mid . (E @ gamma_mid) per sequence; Ln is evaluated as
Ln(tot * 2^-32) on device (the scalar engine's Ln is only valid for
|x| < 2^64 and tot reaches ~2e22) with the 32*ln2 added back on host.

Features are DMA'd per 128-step time chunk in [t, b, n] layout (prefetched
two chunk-pairs ahead), PE-transposed per batch, and exponentiated
PSUM->SBUF by the scalar engine into the [n, b, t] layout the chains
consume.  All prep work for pair k+1 is woven one op per chain slot into
pair k's emission stream so the in-order engines absorb it into the
chains' latency gaps.

The gold-path ("real") score — emission gather, transition pairs, and the
start/end terms — is O(B*T) indexing with no reuse, and is computed on the
host in float64 while the device kernel runs.

Host combine: loss = (sum_log + B*(32*ln2 + T*c) - real_sum) / mask.sum().
"""

import contextlib

import numpy as np

NT = 48          # number of tags
T = 2048         # sequence length
B = 256          # full batch
NCORES = 8
BL = B // NCORES # per-core batch (32)
NG = 2           # chain groups per core
GB = BL // NG    # sequences per group (16)
CT = 128         # time-chunk length
C_BIAS = np.float32(4.3466)  # per-step drift-centering constant

_cached_nc = None


def _build_program(T_=T, BL_=BL, loop_k=1):
    import concourse.bacc as bacc
    import concourse.bass as bass
    import concourse.mybir as mybir
    import concourse.tile as tile
    from concourse._compat import axon_active

    AF = mybir.ActivationFunctionType
    OP = mybir.AluOpType
    dt = mybir.dt
    nchunk = T_ // CT
    npair = nchunk // 2
    GB_ = BL_ // NG
    POFF = 64
    fx_alloc = [0]

    nc = bacc.Bacc("TRN2", target_bir_lowering=False,
                   debug=not axon_active(), num_devices=NCORES)

    feats_d = nc.dram_tensor("feats", [BL_, T_, NT], dt.float32, kind="ExternalInput").ap()
    trans_d = nc.dram_tensor("trans", [NT + 2, NT + 2], dt.float32, kind="ExternalInput").ap()
    ident_d = nc.dram_tensor("ident", [128, 128], dt.float32, kind="ExternalInput").ap()
    out_d = nc.dram_tensor("out", [1, 8], dt.float32, kind="ExternalOutput").ap()
    dbg_d = nc.dram_tensor("dbg", [1, 2 * BL_], dt.float32, kind="ExternalOutput").ap()

    with tile.TileContext(nc) as tc:
        loop_cm = tc.For_i(0, loop_k, 1) if loop_k > 1 else contextlib.nullcontext()
        with (
            loop_cm,
            tc.tile_pool(name="const", bufs=1) as cpool,
            tc.tile_pool(name="fc", bufs=4) as fcpool,
            tc.tile_pool(name="fx", bufs=4) as fxpool,
            tc.tile_pool(name="pairA", bufs=2) as apoolA,
            tc.tile_pool(name="pairB", bufs=2) as apoolB,
            tc.tile_pool(name="fin", bufs=1) as finpool,
            tc.tile_pool(name="psA", bufs=2, space="PSUM") as psA,
            tc.tile_pool(name="psB", bufs=2, space="PSUM") as psB,
            tc.tile_pool(name="pst", bufs=2, space="PSUM") as pst,
        ):
            # ---------------- setup ----------------
            trans_sb = cpool.tile([NT + 2, NT + 2], dt.float32, tag="trans")
            nc.sync.dma_start(trans_sb[:], trans_d[:])
            start_col = cpool.tile([NT, 1], dt.float32, tag="startc")
            nc.sync.dma_start(start_col[:], trans_d[NT:NT + 1, 0:NT].rearrange("a b -> b a"))
            end_col = cpool.tile([NT, 1], dt.float32, tag="endc")
            nc.sync.dma_start(end_col[:], trans_d[0:NT, NT + 1:NT + 2])
            ident = cpool.tile([128, 128], dt.float32, tag="ident")
            nc.sync.dma_start(ident[:], ident_d[:])

            zero48 = cpool.tile([NT, 1], dt.float32, tag="zero48")
            nc.vector.memset(zero48[:], 0.0)
            log48c = cpool.tile([NT, 1], dt.float32, tag="log48c")
            nc.vector.memset(log48c[:], float(np.log(np.float32(NT))))
            negc = cpool.tile([NT, 1], dt.float32, tag="negc")
            nc.vector.memset(negc[:], -float(C_BIAS))
            ones_nt = cpool.tile([NT, 1], dt.float32, tag="ones")
            nc.vector.memset(ones_nt[:], 1.0)

            E32 = cpool.tile([NT, NT], dt.float32, tag="E32")
            nc.scalar.activation(E32[:], trans_sb[0:NT, 0:NT], AF.Exp, bias=zero48[:])
            ps_et = pst.tile([NT, NT], dt.float32, tag="pst")
            nc.tensor.transpose(ps_et[:], E32[:], ident[0:NT, 0:NT])

            # bf16 stationaries: E for the fwd chain, E^T for the bwd chain
            Eb = cpool.tile([NT, NT], dt.bfloat16, tag="Eb")
            nc.scalar.activation(Eb[:], E32[:], AF.Copy)
            Etb = cpool.tile([NT, NT], dt.bfloat16, tag="Etb")
            nc.scalar.activation(Etb[:], ps_et[:], AF.Copy)

            start_e = cpool.tile([NT, 1], dt.float32, tag="starte")
            nc.scalar.activation(start_e[:], start_col[:], AF.Exp, bias=log48c[:])
            e_end = cpool.tile([NT, 1], dt.float32, tag="eend")
            nc.scalar.activation(e_end[:], end_col[:], AF.Exp, bias=zero48[:])

            # ---------------- per-chunk-pair data prep ----------------
            def dma_pair(chp, slabbed=False):
                # Loads the pair's two chunks in [t, b, n] layout.  For the
                # prologue pair (slabbed=True) each chunk is issued as four
                # 8-batch slab DMAs, interleaved fwd/bwd, so the first
                # transposes can start as soon as the first slab lands
                # instead of waiting for the whole 1.6MB load.
                chf, chb = chp, nchunk - 1 - chp
                fcs = []
                for ch in (chf, chb):
                    fc = fcpool.tile([CT, BL_, NT], dt.float32, tag="fc")
                    fcs.append(fc)
                if slabbed:
                    for s in range(0, BL_, 8):
                        for fc, ch in zip(fcs, (chf, chb)):
                            nc.sync.dma_start(
                                fc[:, s:s + 8, :],
                                feats_d[s:s + 8, ch * CT:(ch + 1) * CT, :]
                                .rearrange("b t n -> t b n"))
                else:
                    for fc, ch in zip(fcs, (chf, chb)):
                        nc.sync.dma_start(
                            fc[:],
                            feats_d[:, ch * CT:(ch + 1) * CT, :].rearrange("b t n -> t b n"))
                return fcs

            def fx_ops(fcs):
                """Thunk generator building fxp tiles (natural t order).

                fxs[0]: [48, BL_, CT] bf16 exp(feat) of the fwd chunk;
                fxs[1]: same for the bwd chunk.  The bwd chain indexes
                [:, :, CT-1-t_loc] at slot t_loc — no reversal needed.
                """
                fxs = [fxpool.tile([NT, BL_, CT], dt.bfloat16, tag="fx",
                                   name=f"fx{hi}")
                       for hi in range(2)]

                def gen():
                    for half in (0, 1):          # 0 = fwd chunk, 1 = bwd chunk
                        fc_x = fcs[half]
                        fxp = fxs[half]
                        for q in range(BL_ // 4):
                            ps = pst.tile([NT, 4, CT], dt.float32, tag="pst")
                            for bi in range(4):
                                b = q * 4 + bi
                                yield lambda ps=ps, bi=bi, fc_x=fc_x, b=b: \
                                    nc.tensor.transpose(ps[:, bi, :],
                                                        fc_x[:, b, :], ident[:])
                            dst = fxp[:, q * 4:(q + 1) * 4, :]
                            yield lambda dst=dst, ps=ps: \
                                nc.scalar.activation(dst, ps[:], AF.Exp,
                                                     bias=negc[:])
                return fxs, gen()

            # ---------------- main interleaved chains ----------------
            # group 0 = fwd chain (stationary Eb), group 1 = bwd (Etb)
            pspool = (psA, psB)
            prpool = (apoolA, apoolB)
            W = (Eb, Etb)
            se = (start_e, e_end)
            prev_ps = [None, None]
            pair = [None, None]

            fxs_cur = None
            fcs_cur = dma_pair(0, slabbed=True)
            fxs_cur, gen0 = fx_ops(fcs_cur)
            for op in gen0:   # prologue: build fx for pair 0 upfront
                op()

            for chp in range(npair):
                fillers = []
                if chp + 1 < npair:
                    fcs_nxt = dma_pair(chp + 1)
                    fxs_nxt, gen_fx = fx_ops(fcs_nxt)
                    fillers.append(gen_fx)

                for t_loc in range(CT):
                    s = chp * CT + t_loc
                    tidx = (t_loc, CT - 1 - t_loc)
                    for g in range(2):
                        pr = prpool[g].tile([NT, BL_], dt.bfloat16, tag="pair",
                                            name=f"pr{g}")
                        if s == 0:
                            nc.vector.tensor_tensor(
                                pr[:], fxs_cur[g][:, :, tidx[g]],
                                se[g][:].broadcast_to([NT, BL_]), OP.mult)
                        else:
                            nc.vector.tensor_tensor(
                                pr[:], prev_ps[g][:],
                                fxs_cur[g][:, :, tidx[g]], OP.mult)
                        pair[g] = pr
                    for g in range(2):
                        psp = pspool[g].tile([NT, BL_], dt.float32, tag="pp",
                                             name=f"pp{g}")
                        nc.tensor.matmul(psp[:], lhsT=W[g][:], rhs=pair[g][:],
                                         start=True, stop=True)
                        prev_ps[g] = psp

                    # weave ~1 filler op per slot
                    for q_ in list(fillers):
                        try:
                            next(q_)()
                        except StopIteration:
                            fillers.remove(q_)

                for q_ in list(fillers):
                    for op in q_:
                        op()
                if chp + 1 < npair:
                    fcs_cur, fxs_cur = fcs_nxt, fxs_nxt

            # ---------------- final ----------------
            # prev_ps[1] = E @ gamma_{mid+1}; pair[0] = alpha_mid
            out_sb = finpool.tile([1, 8], dt.float32, tag="outsb")
            nc.vector.memset(out_sb[:], 0.0)
            prod = finpool.tile([NT, BL_], dt.float32, tag="prod")
            nc.vector.tensor_tensor(prod[:], prev_ps[1][:], pair[0][:], OP.mult)
            ps_fin = pspool[0].tile([NT, BL_], dt.float32, tag="pp", name="ppf")
            nc.tensor.matmul(ps_fin[0:1, :], lhsT=ones_nt[:], rhs=prod[:],
                             start=True, stop=True)
            # scale by 2^-32 before Ln: tot can exceed the scalar engine's
            # Ln-valid range (|x| < 2^64); host adds back 32*ln(2) per seq.
            logs = finpool.tile([1, BL_], dt.float32, tag="logs")
            nc.scalar.activation(logs[:], ps_fin[0:1, :], AF.Ln,
                                 bias=zero48[0:1, :], scale=float(2.0 ** -32))
            nc.vector.tensor_reduce(out_sb[:, 0:1], logs[:],
                                    mybir.AxisListType.X, OP.add)

            dbg_sb = finpool.tile([1, 2 * BL_], dt.float32, tag="dbgsb")
            nc.scalar.activation(dbg_sb[:, 0:BL_], ps_fin[0:1, :], AF.Copy)
            nc.scalar.activation(dbg_sb[:, BL_:2 * BL_], logs[:], AF.Copy)
            nc.sync.dma_start(dbg_d[:], dbg_sb[:])
            nc.sync.dma_start(out_d[:], out_sb[:])

    nc.compile()
    return nc


def _get_nc():
    global _cached_nc
    if _cached_nc is None:
        _cached_nc = _build_program()
    return _cached_nc


def _make_consts():
    ident = np.eye(128, dtype=np.float32)
    return ident


def _host_gold(feats, trans, tags_):
    """Real-path score sum over batch, float64 (exact)."""
    tagsl = np.asarray(tags_).astype(np.int64)
    em = np.take_along_axis(feats, tagsl[..., None], axis=2)[..., 0]
    em_sum = em.sum(dtype=np.float64)
    tr = trans[:NT, :NT]
    pair_sum = tr[tagsl[:, :-1], tagsl[:, 1:]].sum(dtype=np.float64)
    first = np.float64(trans[NT, tagsl[:, 0]].sum(dtype=np.float64))
    last = np.float64(trans[tagsl[:, -1], NT + 1].sum(dtype=np.float64))
    return em_sum + pair_sum + first + last


def _numpy_fallback(inputs, transitions, output_mask, tags):
    """Reference semantics in numpy; only used if mask is not all-ones."""
    feats = np.asarray(inputs, np.float32)
    trans = np.asarray(transitions, np.float32)
    mask = np.asarray(output_mask).astype(np.float32)
    tags_ = np.asarray(tags).astype(np.int64)
    Bs, Tl, Ntag = feats.shape
    start, end = Ntag, Ntag + 1
    lengths = np.asarray(output_mask).sum(axis=1)
    tr = trans[:Ntag, :Ntag]
    em = np.take_along_axis(feats, tags_[..., None], axis=2)[..., 0]
    em_score = (em * mask).sum(axis=1)
    first = trans[start, tags_[:, 0]]
    pair = tr[tags_[:, :-1], tags_[:, 1:]]
    pair_score = (pair * mask[:, 1:]).sum(axis=1)
    last_tag = np.take_along_axis(tags_, (lengths - 1)[:, None], axis=1)[:, 0]
    real = em_score + first + pair_score + trans[last_tag, end]

    fwd = feats[:, 0, :] + trans[start, :Ntag][None, :] + np.log(np.float32(Ntag))
    for t in range(1, Tl):
        s = fwd[:, :, None] + tr[None, :, :]
        mx = s.max(axis=1)
        new = mx + np.log(np.exp(s - mx[:, None, :]).sum(axis=1)) + feats[:, t, :]
        keep = (t < lengths)[:, None]
        fwd = np.where(keep, new, fwd)
    v = fwd + trans[:Ntag, end][None, :]
    mx = v.max(axis=1)
    total = mx + np.log(np.exp(v - mx[:, None]).sum(axis=1))
    return np.float32((total - real).sum() / mask.sum())


def kernel(inputs, transitions, output_mask, tags):
    feats = np.ascontiguousarray(np.asarray(inputs, dtype=np.float32))
    trans = np.ascontiguousarray(np.asarray(transitions, dtype=np.float32))
    mask = np.asarray(output_mask)
    tags_ = np.asarray(tags)

    if not bool((np.asarray(mask) == 1).all()):
        return _numpy_fallback(inputs, transitions, output_mask, tags)

    ident = _make_consts()

    from concourse.bass_utils import run_bass_kernel_spmd

    nc = _get_nc()
    in_maps = []
    for c in range(NCORES):
        sl = slice(c * BL, (c + 1) * BL)
        in_maps.append({
            "feats": feats[sl],
            "trans": trans,
            "ident": ident,
        })
    res = run_bass_kernel_spmd(nc, in_maps, core_ids=list(range(NCORES)))
    outs = [np.asarray(r["out"], np.float64).reshape(-1) for r in res.results]

    sum_log = sum(o[0] for o in outs)
    real_sum = _host_gold(feats, trans, tags_)
    num_chars = np.float64(np.asarray(mask, np.int64).sum())

    # device computes Ln(tot * 2^-32); add back 32*ln2 per sequence
    total_sum = (sum_log + np.float64(B) * 32.0 * np.log(np.float64(2.0))
                 + np.float64(B) * np.float64(T) * np.float64(C_BIAS))
    loss = (total_sum - real_sum) / num_chars
    return np.float32(loss)
